# revision 23
# baseline (speedup 1.0000x reference)
"""Causal FFT-conv (B=32, Cin=Cout=128, L=K=4096) on 8 trn2 NeuronCores.

Pipeline (wire-byte minimized; the axon tunnel runs ~60-80 MB/s so
transfer dominates):
  host: rfft(x, 8192), rfft(w, 8192) via scipy (float32-preserving);
        slice spectra by frequency across the 8 cores, cast to bf16,
        upload asynchronously in chunks so FFT/cast overlaps transfer.
  device (per core, 513 bins): for each bin f the complex channel
        contraction  yhat[b,o] = sum_c xhat[c,b] * conj(what)[c,o]
        as TWO bf16 matmuls into one PSUM tile [32, 256] = (yr | yi):
          mm1: stat A=ReX [c,32], stream [Q|Pn] -> (A.Q | -A.P)
          mm2: stat B=ImX [c,32], stream [P|Q]  -> (B.P |  B.Q)
        with P=Im(W), Q=Re(W), Pn=-P (negated on device).  The padding
        shift twiddles (x left-pad 4096, w left-pad 1) are folded into
        a per-bin factor t[f] applied on host to the returned spectrum.
  host: assemble yhat, *= t, bias into bin 0, irfft, crop to L.
"""

import os
import sys
import time

sys.path.insert(0, "/opt/trn_rl_repo")

import numpy as np

_T0 = time.time()


def _tp(msg):
    if os.environ.get("KPROF"):
        print(f"[kprof {time.time() - _T0:6.2f}s] {msg}", flush=True)

B, C, O, L = 32, 128, 128, 4096
N = 8192
F = N // 2 + 1          # 4097 rfft bins
NCORES = 8
FC = 520                # bins per core (8*520 = 4160 >= 4097, zero padded)
FH = 260                # bins per half-dispatch; 2 halves of 13 blocks each
NFB = 20                # bins per inner block; 13 * 20 = 260
WIN = 448               # SBUF cols per bin: [A 32 | B 32 | P 128 | Q 128 | Pn 128]
OCHUNKS = [16, 16, 32, 32, 32]  # o-chunk sizes; small first chunk starts the tunnel sooner
NOCH = len(OCHUNKS)

last_exec_ns = None
_cache = {}
_init_ready = None


def _start_jax_init():
    """Kick off the axon device handshake (~0.5-1s of I/O) in the
    background; runs at import so it overlaps the caller's input prep."""
    global _init_ready
    if _init_ready is not None:
        return _init_ready
    import threading

    ev = threading.Event()

    def _init():
        try:
            import jax
            from jax.sharding import Mesh, NamedSharding, PartitionSpec

            devices = jax.devices()[:NCORES]
            _cache["mesh"] = Mesh(np.asarray(devices), ("core",))
            _cache["sharding"] = NamedSharding(
                _cache["mesh"], PartitionSpec("core")
            )
        finally:
            ev.set()

    threading.Thread(target=_init, daemon=True).start()
    _init_ready = ev
    return ev


_start_jax_init()


def _build_bass():
    from concourse import bass, bacc, mybir
    from concourse.tile import TileContext

    dt = mybir.dt
    nc = bacc.Bacc(None, target_bir_lowering=False)

    xc = nc.dram_tensor("xc", [B, 2, C, FH], dt.bfloat16, kind="ExternalInput")
    wc = [
        nc.dram_tensor(f"wc{k}", [OCHUNKS[k], 2, C, FH], dt.bfloat16, kind="ExternalInput")
        for k in range(NOCH)
    ]
    xre, xim = xc[:, 0], xc[:, 1]
    wim = [wc[k][:, 0] for k in range(NOCH)]
    wre = [wc[k][:, 1] for k in range(NOCH)]
    y = nc.dram_tensor("y", [B, 2 * O, FH], dt.bfloat16, kind="ExternalOutput")

    OBASE = [sum(OCHUNKS[:k]) for k in range(NOCH)]
    with TileContext(nc) as tc:
        with (
            tc.tile_pool(name="xin", bufs=2) as xpool,
            tc.tile_pool(name="stg", bufs=2) as spool,
            tc.tile_pool(name="yout", bufs=3) as ypool,
            tc.tile_pool(name="ps", bufs=4, space="PSUM") as pspool,
        ):
            for blk in range(FH // NFB):
                f0 = blk * NFB
                fsl = slice(f0, f0 + NFB)
                # Stage DRAM->SBUF keeping contiguous f-runs innermost
                # (DMA needs a shared contiguous final dim, <=3 dims); the
                # transpose into per-bin windows happens on vector engine.
                xst = spool.tile([C, 2 * B * NFB], dt.bfloat16, tag="xst")
                xsr = xst.rearrange("c (b f) -> c b f", f=NFB)
                nc.gpsimd.dma_start(
                    out=xsr[:, 0:B], in_=xre[:, :, fsl].rearrange("b c f -> c b f")
                )
                nc.gpsimd.dma_start(
                    out=xsr[:, B : 2 * B],
                    in_=xim[:, :, fsl].rearrange("b c f -> c b f"),
                )
                wstp = spool.tile([C, O * NFB], dt.bfloat16, tag="wstp")
                wstq = spool.tile([C, O * NFB], dt.bfloat16, tag="wstq")
                wpr = wstp.rearrange("c (o f) -> c o f", f=NFB)
                wqr = wstq.rearrange("c (o f) -> c o f", f=NFB)
                for k in range(NOCH):
                    osl = slice(OBASE[k], OBASE[k] + OCHUNKS[k])
                    nc.gpsimd.dma_start(
                        out=wpr[:, osl],
                        in_=wim[k][:, :, fsl].rearrange("o c f -> c o f"),
                    )
                    nc.gpsimd.dma_start(
                        out=wqr[:, osl],
                        in_=wre[k][:, :, fsl].rearrange("o c f -> c o f"),
                    )

                xt = xpool.tile([C, NFB * WIN], dt.bfloat16, tag="x")
                xtr = xt.rearrange("c (f z) -> c f z", f=NFB)
                # window per bin: [A 0:32 | B 32:64 | P 64:192 | Q 192:320 | Pn 320:448]
                nc.vector.tensor_copy(
                    xtr[:, :, 0 : 2 * B], xst.rearrange("c (b f) -> c f b", f=NFB)
                )
                nc.vector.tensor_copy(
                    xtr[:, :, 64:192], wstp.rearrange("c (o f) -> c f o", f=NFB)
                )
                nc.vector.tensor_copy(
                    xtr[:, :, 192:320], wstq.rearrange("c (o f) -> c f o", f=NFB)
                )
                nc.vector.tensor_scalar_mul(
                    xtr[:, :, 320:448],
                    wstp.rearrange("c (o f) -> c f o", f=NFB),
                    -1.0,
                )

                # yo col = z*NFB + f  (z = r*O + o), so f stays contiguous
                yo = ypool.tile([B, 2 * O * NFB], dt.bfloat16, tag="yo")
                yor = yo.rearrange("b (z f) -> b z f", f=NFB)
                for p in range(0, NFB, 2):
                    gn = min(2, NFB - p)
                    ps = pspool.tile([B, gn * 256], dt.float32, tag="ps")
                    for j in range(gn):
                        wb = (p + j) * WIN
                        sl = slice(j * 256, (j + 1) * 256)
                        # (A.Q | -A.P) + (B.P | B.Q) -> (yr | yi)
                        nc.tensor.matmul(
                            ps[:, sl], xt[:, wb : wb + 32],
                            xt[:, wb + 192 : wb + 448], start=True, stop=False,
                        )
                        nc.tensor.matmul(
                            ps[:, sl], xt[:, wb + 32 : wb + 64],
                            xt[:, wb + 64 : wb + 320], start=False, stop=True,
                        )
                    nc.vector.tensor_copy(
                        yor[:, :, p : p + gn],
                        ps.rearrange("b (f z) -> b z f", f=gn),
                    )
                nc.gpsimd.dma_start(
                    out=y[:, :, fsl], in_=yor
                )
    nc.compile()
    return nc


def _make_runner(nc):
    """Vendored from bass2jax.run_bass_via_pjrt: same custom-call path, but
    accepts pre-committed sharded device arrays (so uploads overlap host
    work) and returns the device output array without blocking."""
    import jax
    from jax.sharding import Mesh, PartitionSpec
    from jax.experimental.shard_map import shard_map
    from concourse import bass2jax, mybir

    bass2jax.install_neuronx_cc_hook()

    partition_name = nc.partition_id_tensor.name if nc.partition_id_tensor else None
    in_names, out_names, out_avals, out_shapes = [], [], [], []
    for alloc in nc.m.functions[0].allocations:
        if type(alloc).__name__ != "MemoryLocationSet":
            continue
        name = alloc.memorylocations[0].name
        if alloc.kind == "ExternalInput":
            if name != partition_name:
                in_names.append(name)
        elif alloc.kind == "ExternalOutput":
            shape = tuple(alloc.tensor_shape)
            dtype = mybir.dt.np(alloc.dtype)
            out_names.append(name)
            out_avals.append(jax.core.ShapedArray(shape, dtype))
            out_shapes.append((shape, dtype))
    n_params = len(in_names)
    all_names = in_names + out_names
    if partition_name is not None:
        all_names = all_names + [partition_name]
    donate = tuple(range(n_params, n_params + len(out_names)))

    def _body(*args):
        operands = list(args)
        if partition_name is not None:
            operands.append(bass2jax.partition_id_tensor())
        outs = bass2jax._bass_exec_p.bind(
            *operands,
            out_avals=tuple(out_avals),
            in_names=tuple(all_names),
            out_names=tuple(out_names),
            lowering_input_output_aliases=(),
            sim_require_finite=True,
            sim_require_nnan=True,
            nc=nc,
        )
        return tuple(outs)

    devices = jax.devices()[:NCORES]
    mesh = Mesh(np.asarray(devices), ("core",))
    nargs = n_params + len(out_names)
    sharded = jax.jit(
        shard_map(
            _body,
            mesh=mesh,
            in_specs=(PartitionSpec("core"),) * nargs,
            out_specs=(PartitionSpec("core"),) * len(out_names),
            check_rep=False,
        ),
        donate_argnums=donate,
        keep_unused=True,
    )
    return sharded, in_names, out_names, out_shapes, mesh


def kernel(x: np.ndarray, weight: np.ndarray, bias: np.ndarray) -> np.ndarray:
    import ml_dtypes
    import scipy.fft as sf
    import jax

    bf16 = ml_dtypes.bfloat16
    x = np.asarray(x, np.float32)
    weight = np.asarray(weight, np.float32)
    bias = np.asarray(bias, np.float32)

    ready = _start_jax_init()

    putsA, keepB, putsB = {}, {}, {}

    def pack_halves(spec_im, spec_re, rows):
        # per core r, half h: global bins [r*FC + h*FH, +FH) -> [rows,2,C,FH]
        gA = np.zeros((NCORES * rows, 2, C, FH), bf16)
        gB = np.zeros((NCORES * rows, 2, C, FH), bf16)
        for r in range(NCORES):
            for h, g in ((0, gA), (1, gB)):
                lo = r * FC + h * FH
                sl = slice(lo, min(lo + FH, F))
                n = sl.stop - sl.start
                if n <= 0:
                    continue
                rs = slice(rows * r, rows * r + rows)
                g[rs, 0, :, :n] = spec_im[:, :, sl].astype(bf16)
                g[rs, 1, :, :n] = spec_re[:, :, sl].astype(bf16)
        return gA, gB

    def putA(name, gA, gB):
        putsA[name] = jax.device_put(gA, _cache["sharding"])
        keepB[name] = gB

    # --- weight spectrum, chunked over o so cast overlaps upload ---
    OBASE = [sum(OCHUNKS[:k]) for k in range(NOCH)]
    for k in range(NOCH):
        _tp(f"w chunk {k} fft start")
        oc = OCHUNKS[k]
        wf = sf.rfft(weight[OBASE[k] : OBASE[k] + oc], n=N, axis=-1)  # [oc,C,F] c64
        gA, gB = pack_halves(wf.imag, wf.real, oc)
        if k == 0:
            ready.wait()
            if "sharding" not in _cache:
                raise RuntimeError("jax/axon device initialization failed")
        _tp(f"w chunk {k} put")
        putA(f"wc{k}", gA, gB)

    # --- x spectrum (packed order: [re, im] -> comp 0/1) ---
    _tp("x fft start")
    xf = sf.rfft(x, n=N, axis=-1)  # [B,C,F] c64
    gA, gB = pack_halves(xf.real, xf.imag, B)
    _tp("x put")
    putA("xc", gA, gB)

    # donated zero output buffers + B-half uploads, then build/compile --
    # all of it streams while half A's exec/fetch proceed (tunnel is duplex)
    _tp("zeros put")
    zeros = np.zeros((NCORES * B, 2 * O, FH), bf16)
    zeroA = jax.device_put(zeros, _cache["sharding"])
    _tp("puts B")
    for name, gB in keepB.items():
        putsB[name] = jax.device_put(gB, _cache["sharding"])
    zeroB = jax.device_put(zeros, _cache["sharding"])
    _tp("build bass start")
    if "nc" not in _cache:
        _cache["nc"] = _build_bass()
    _tp("make runner start")
    if "runner" not in _cache:
        _cache["runner"] = _make_runner(_cache["nc"])
    sharded, in_names, out_names, out_shapes, mesh = _cache["runner"]

    _tp("dispatch jit A")
    outA = sharded(*([putsA[n] for n in in_names] + [zeroA]))
    _tp("dispatch jit B")
    outB = sharded(*([putsB[n] for n in in_names] + [zeroB]))
    _tp("jits dispatched; fetching")

    # --- host: assemble spectrum per fetched shard, twiddle, bias, irfft ---
    yc = np.empty((B, O, NCORES * FC), np.complex64)
    for h, arrs in ((0, outA), (1, outB)):
        shards = sorted(
            arrs[0].addressable_shards, key=lambda s: s.index[0].start or 0
        )
        for r, sh in enumerate(shards):
            yg = np.asarray(sh.data)  # [B, 2*O, FH] bf16
            ygf = yg.astype(np.float32).reshape(B, 2, O, FH)
            lo = r * FC + h * FH
            yc.real[:, :, lo : lo + FH] = ygf[:, 0]
            yc.imag[:, :, lo : lo + FH] = ygf[:, 1]
        _tp(f"half {h} fetched")
    _tp("fetched+assembled")
    # Free device buffers eagerly: deallocating ~500MB x 8 cores lazily at
    # interpreter exit stalls whatever session starts next on this tunnel.
    for a in list(outA) + list(outB):
        a.delete()
    for a in list(putsA.values()) + list(putsB.values()):
        a.delete()
    yv = yc[:, :, :F]
    # fold out the causal left-pad shifts: x by K-1=4096 -> (-1)^f, w by 1
    tw = np.exp(1j * np.pi * np.arange(F) * (N // 2 + 1) / (N // 2)).astype(
        np.complex64
    )
    yv *= tw
    yv[:, :, 0] += (bias * np.float32(N)).astype(np.float32)[None, :]
    out = sf.irfft(yv, n=N, axis=-1)[:, :, :L]
    _tp("done")
    return np.ascontiguousarray(out, dtype=np.float32)
